# revision 7
# baseline (speedup 1.0000x reference)
"""Trainium2 Bass kernel for a dense transformer layer (attention + FFN + 2 LayerNorms).

Problem shapes: x [4, 2048, 1024], d_model=1024, heads=16 (hd=64), d_ff=4096.

Sharding: 8 cores; core c handles batch b = c//2, sequence half = c%2
(1024 query tokens).  Each core computes K/V for its batch's full 2048
tokens (duplicated across the pair — avoids any cross-core communication).
The host permutes each core's x so its own token half comes first; since
attention sums over key tokens, K/V token order is irrelevant as long as
K and V agree.

Layout strategy: activations are feature-major ("transposed": d_model on
partitions, tokens on free dim) so weight matrices serve directly as
matmul stationary operands (out = lhsT.T @ rhs).  Attention computes
S^T = K @ Q^T per head (key tokens on partitions), exp on the scalar
engine, then ctx^T = V_aug^T @ P^T where V carries a ones-column that
yields the softmax denominator for free.  All matmuls use float32r
(full-rate fp32 PE streaming, ~1e-4 relative rounding).

K^T, V (token-major) and ctx^T bounce through HBM to bound SBUF pressure.
"""

import os
import numpy as np

import concourse.bass as bass
import concourse.tile as tile
from concourse import bacc, mybir
from concourse import bass_utils

F32R = mybir.dt.float32r
F32 = mybir.dt.float32
AF = mybir.ActivationFunctionType
OP = mybir.AluOpType

D = 1024          # d_model
S = 2048          # full sequence per batch
T = 1024          # query tokens per core
H = 16            # heads
HD = 64           # head dim
F = 4096          # ffn hidden
P = 128
DT = D // P       # 8 feature tiles
KT = S // P       # 16 key-token tiles
FT = F // P       # 32 hidden tiles
N_CORES = 8
EPS = 1e-5

_CACHED = {}


def _build_program():
    nc = bacc.Bacc("TRN2", target_bir_lowering=False, debug=False,
                   num_devices=N_CORES)

    tens = {}

    def di(name, shape, dtype=F32R):
        tens[name] = nc.dram_tensor(name, shape, dtype, kind="ExternalInput")

    di("xT", [D, S])
    di("wq", [D, D]); di("wk", [D, D]); di("wv", [D, D]); di("wo", [D, D])
    di("w1", [D, F]); di("w2", [F, D])
    for nm in ["bq_p", "bk_p", "bv_p", "bo_p", "b2_p", "g1_p", "be1_p"]:
        di(nm, [P, DT], F32)
    di("b1_p", [P, FT], F32)
    di("g2_d", [D], F32); di("be2_d", [D], F32)
    di("ident_d", [P, P]); di("ones_row_d", [1, P]); di("ones_col_d", [P, 1])
    di("ones16_d", [P, H, 1])
    tens["out"] = nc.dram_tensor("out", [T, D], F32, kind="ExternalOutput")

    with tile.TileContext(nc) as tc:
        _trace_kernel(nc, tc, tens)
    nc.compile()
    return nc


def _trace_kernel(nc, tc, t):
    xT, wq, wk, wv, wo, w1, w2 = (t["xT"], t["wq"], t["wk"], t["wv"], t["wo"],
                                  t["w1"], t["w2"])
    out = t["out"]

    from contextlib import ExitStack
    es = ExitStack()
    with es:
        dram = es.enter_context(tc.tile_pool(name="dram", bufs=1, space="DRAM"))
        kT_hbm = dram.tile([D, S], F32R, tag="kh", name="kh")
        v_hbm = dram.tile([S, H, HD + 1], F32R, tag="vh", name="vh")
        ctx_hbm = dram.tile([D, T], F32R, tag="ch", name="ch")

        const = es.enter_context(tc.tile_pool(name="const", bufs=1))
        ident = const.tile([P, P], F32R, tag="ident", name="ident")
        nc.sync.dma_start(out=ident, in_=t["ident_d"][:, :])
        ones_row = const.tile([1, P], F32R, tag="onesr", name="onesr")
        nc.sync.dma_start(out=ones_row, in_=t["ones_row_d"][:, :])
        ones_col = const.tile([P, 1], F32R, tag="onesc", name="onesc")
        nc.sync.dma_start(out=ones_col, in_=t["ones_col_d"][:, :])
        ones16 = const.tile([P, H, 1], F32R, tag="ones16", name="ones16")
        nc.sync.dma_start(out=ones16, in_=t["ones16_d"][:, :, :])
        biases = {}
        for name in ["bq_p", "bk_p", "bv_p", "bo_p", "b2_p", "g1_p", "be1_p"]:
            bt = const.tile([P, DT], F32, tag=name)
            nc.sync.dma_start(out=bt, in_=t[name][:, :])
            biases[name] = bt
        eps_sb = const.tile([P, 1], F32, tag="eps", name="eps")
        nc.vector.memset(eps_sb[:], EPS)
        b1_sb = const.tile([P, FT], F32, tag="b1", name="b1")
        nc.sync.dma_start(out=b1_sb, in_=t["b1_p"][:, :])
        hT_pool = es.enter_context(tc.tile_pool(name="hT", bufs=1))
        hT = [hT_pool.tile([P, T], F32R, tag=f"hT{i}", name=f"hT{i}") for i in range(DT)]

        # =============== Phase 1: QKV projections =======================
        qt_cm = tc.tile_pool(name="qt", bufs=1)
        qt_pool = qt_cm.__enter__()
        QT = [qt_pool.tile([P, T], F32R, tag=f"qt{i}", name=f"qt{i}") for i in range(DT)]

        with tc.tile_pool(name="xsb", bufs=1) as xsb_pool, \
             tc.tile_pool(name="wsb", bufs=DT) as wsb_pool, \
             tc.tile_pool(name="p1ev", bufs=2) as ev_pool, \
             tc.tile_pool(name="vsb", bufs=4) as vsb_pool, \
             tc.tile_pool(name="psA", bufs=2, space="PSUM") as psA, \
             tc.tile_pool(name="psT", bufs=4, space="PSUM") as psT:
            xsb = []
            for dt_ in range(DT):
                xt_ = xsb_pool.tile([P, S], F32R, tag=f"x{dt_}", name=f"x{dt_}")
                nc.sync.dma_start(out=xt_, in_=xT[dt_ * P:(dt_ + 1) * P, :])
                xsb.append(xt_)

            def proj_psum(ps, w_sb, dout, cols):
                for din in range(DT):
                    nc.tensor.matmul(
                        ps[:], w_sb[din][:, dout * P:(dout + 1) * P],
                        xsb[din][:, cols], start=(din == 0),
                        stop=(din == DT - 1))

            def load_w(wd):
                w_sb = [wsb_pool.tile([P, D], F32R, tag="w", name="w") for _ in range(DT)]
                for dt_ in range(DT):
                    nc.sync.dma_start(out=w_sb[dt_],
                                      in_=wd[dt_ * P:(dt_ + 1) * P, :])
                return w_sb

            # --- V projection -> transpose -> v_hbm ---------------------
            wv_sb = load_w(wv)
            for ch in range(S // 512):           # 4 chunks of 512 tokens
                vtiles = [vsb_pool.tile([P, H, HD + 1], F32R, tag="vsb", name="vsb")
                          for _ in range(4)]
                for dout in range(DT):
                    ps = psA.tile([P, 512], F32, tag="psA", name="psA")
                    proj_psum(ps, wv_sb, dout, slice(ch * 512, (ch + 1) * 512))
                    vt = ev_pool.tile([P, 512], F32R, tag="vt", name="vt")
                    nc.scalar.activation(vt[:], ps[:], AF.Identity,
                                         bias=biases["bv_p"][:, dout:dout + 1])
                    for hh in range(2):
                        h = 2 * dout + hh
                        idsl = ident[hh * HD:(hh + 1) * HD,
                                     hh * HD:(hh + 1) * HD]
                        for st in range(4):
                            pt = psT.tile([P, HD], F32R, tag="psT", name="psT")
                            nc.tensor.transpose(
                                pt[:], vt[hh * HD:(hh + 1) * HD,
                                          st * P:(st + 1) * P],
                                idsl)
                            nc.vector.tensor_copy(vtiles[st][:, h, 0:HD],
                                                  pt[:])
                for st in range(4):
                    nc.vector.tensor_copy(vtiles[st][:, :, HD:HD + 1],
                                          ones16[:, :, :])
                    kt0 = ch * 4 + st
                    nc.sync.dma_start(
                        out=v_hbm[kt0 * P:(kt0 + 1) * P, :, :],
                        in_=vtiles[st][:, :, :])

            # --- K projection -> kT_hbm ---------------------------------
            wk_sb = load_w(wk)
            for ch in range(S // 512):
                for dout in range(DT):
                    ps = psA.tile([P, 512], F32, tag="psA", name="psA")
                    proj_psum(ps, wk_sb, dout, slice(ch * 512, (ch + 1) * 512))
                    kt_ = ev_pool.tile([P, 512], F32R, tag="kt", name="kt")
                    nc.scalar.activation(kt_[:], ps[:], AF.Identity,
                                         bias=biases["bk_p"][:, dout:dout + 1])
                    nc.sync.dma_start(
                        out=kT_hbm[dout * P:(dout + 1) * P,
                                   ch * 512:(ch + 1) * 512],
                        in_=kt_[:])

            # --- Q projection (own tokens = xT cols 0:1024; resident) ---
            wq_sb = load_w(wq)
            for ch in range(T // 512):           # 2 chunks
                for dout in range(DT):
                    ps = psA.tile([P, 512], F32, tag="psA", name="psA")
                    proj_psum(ps, wq_sb, dout, slice(ch * 512, (ch + 1) * 512))
                    nc.scalar.activation(QT[dout][:, ch * 512:(ch + 1) * 512],
                                         ps[:], AF.Identity,
                                         bias=biases["bq_p"][:, dout:dout + 1])

        # =============== Phase 2: attention =============================
        with tc.tile_pool(name="kbuf", bufs=2) as kbuf_pool, \
             tc.tile_pool(name="vbuf", bufs=2 * KT) as vbuf_pool, \
             tc.tile_pool(name="pbuf", bufs=3) as pbuf_pool, \
             tc.tile_pool(name="cev", bufs=3) as cev_pool, \
             tc.tile_pool(name="psS", bufs=3, space="PSUM") as psS, \
             tc.tile_pool(name="psC", bufs=2, space="PSUM") as psC, \
             tc.tile_pool(name="psB", bufs=2, space="PSUM") as psB:
            for h in range(H):
                ksb = kbuf_pool.tile([HD, S], F32R, tag="kb", name="kb")
                nc.sync.dma_start(out=ksb, in_=kT_hbm[h * HD:(h + 1) * HD, :])
                vsb = [vbuf_pool.tile([P, HD + 1], F32R, tag="vb", name="vb")
                       for _ in range(KT)]
                for j in range(KT):
                    nc.sync.dma_start(out=vsb[j],
                                      in_=v_hbm[j * P:(j + 1) * P, h, :])
                dt_ = h // 2
                r0 = (h % 2) * HD
                for qc in range(T // 512):
                    # stage the head's Q rows at base partition 0 (matmul
                    # requires equal base partitions for lhsT and rhs)
                    qstg = cev_pool.tile([HD, 512], F32R, tag="qstg",
                                         name="qstg")
                    nc.vector.tensor_copy(
                        qstg[:], QT[dt_][r0:r0 + HD, qc * 512:(qc + 1) * 512])
                    cps = psC.tile([HD + 1, 512], F32, tag="cps", name="cps")
                    for j in range(KT):
                        sps = psS.tile([P, 512], F32, tag="sps", name="sps")
                        nc.tensor.matmul(sps[:], ksb[:, j * P:(j + 1) * P],
                                         qstg[:], start=True, stop=True)
                        pT = pbuf_pool.tile([P, 512], F32R, tag="pT", name="pT")
                        nc.scalar.activation(pT[:], sps[:], AF.Exp)
                        nc.tensor.matmul(cps[:], vsb[j][:], pT[:],
                                         start=(j == 0), stop=(j == KT - 1))
                    # normalize: reciprocal of denom row, broadcast via PE
                    recip = cev_pool.tile([1, 512], F32, tag="recip", name="recip")
                    nc.vector.reciprocal(recip[:], cps[HD:HD + 1, :])
                    recip_r = cev_pool.tile([1, 512], F32R, tag="recipr", name="recipr")
                    nc.vector.tensor_copy(recip_r[:], recip[:])
                    bps = psB.tile([HD, 512], F32, tag="bps", name="bps")
                    nc.tensor.matmul(bps[:], ones_row[:, 0:HD], recip_r[:],
                                     start=True, stop=True)
                    bcs = cev_pool.tile([HD, 512], F32, tag="bcs", name="bcs")
                    nc.scalar.copy(bcs[:], bps[:])
                    ctx_sb = cev_pool.tile([HD, 512], F32R, tag="ctxe", name="ctxe")
                    nc.vector.scalar_tensor_tensor(
                        ctx_sb[:], cps[0:HD, :], 1.0, bcs[:],
                        op0=OP.mult, op1=OP.mult)
                    nc.sync.dma_start(
                        out=ctx_hbm[h * HD:(h + 1) * HD,
                                    qc * 512:(qc + 1) * 512],
                        in_=ctx_sb[:])

        qt_cm.__exit__(None, None, None)

        # =============== Phase 3: Wo + residual + LN1 ===================
        with tc.tile_pool(name="ctxs", bufs=1) as ctxs_pool, \
             tc.tile_pool(name="xq", bufs=1) as xq_pool, \
             tc.tile_pool(name="wos", bufs=DT) as wos_pool, \
             tc.tile_pool(name="zT", bufs=1) as zT_pool, \
             tc.tile_pool(name="ln1", bufs=1) as ln1_pool, \
             tc.tile_pool(name="psW", bufs=3, space="PSUM") as psW, \
             tc.tile_pool(name="psStat", bufs=1, space="PSUM") as psStat, \
             tc.tile_pool(name="psBc", bufs=1, space="PSUM") as psBc:
            ctxs = [ctxs_pool.tile([P, T], F32R, tag=f"ctx{i}", name=f"ctx{i}")
                    for i in range(DT)]
            for dt_ in range(DT):
                nc.sync.dma_start(out=ctxs[dt_],
                                  in_=ctx_hbm[dt_ * P:(dt_ + 1) * P, :])
            xq = [xq_pool.tile([P, T], F32, tag=f"xq{i}", name=f"xq{i}") for i in range(DT)]
            for dt_ in range(DT):
                nc.sync.dma_start(
                    out=xq[dt_],
                    in_=xT[dt_ * P:(dt_ + 1) * P, 0:T].bitcast(F32))
            wo_sb = [wos_pool.tile([P, D], F32R, tag="wo", name="wo") for _ in range(DT)]
            for dt_ in range(DT):
                nc.sync.dma_start(out=wo_sb[dt_],
                                  in_=wo[dt_ * P:(dt_ + 1) * P, :])
            zT = [zT_pool.tile([P, T], F32R, tag=f"zT{i}", name=f"zT{i}") for i in range(DT)]
            for ch in range(T // 512):
                for dout in range(DT):
                    ps = psW.tile([P, 512], F32, tag="psW", name="psW")
                    for din in range(DT):
                        nc.tensor.matmul(
                            ps[:], wo_sb[din][:, dout * P:(dout + 1) * P],
                            ctxs[din][:, ch * 512:(ch + 1) * 512],
                            start=(din == 0), stop=(din == DT - 1))
                    # z = attn_out + bo + x_resid
                    nc.vector.scalar_tensor_tensor(
                        zT[dout][:, ch * 512:(ch + 1) * 512], ps[:],
                        biases["bo_p"][:, dout:dout + 1],
                        xq[dout][:, ch * 512:(ch + 1) * 512],
                        op0=OP.add, op1=OP.add)

            # ---- LN1 (feature-major; stats over partitions via PE) -----
            for ch in range(T // 512):
                sl = slice(ch * 512, (ch + 1) * 512)
                sum_ps = psStat.tile([1, 512], F32, tag="s", name="s")
                sq_ps = psStat.tile([1, 512], F32, tag="q", name="q")
                for dt_ in range(DT):
                    zsq = ln1_pool.tile([P, 512], F32R, tag="zsq", name="zsq")
                    nc.vector.tensor_tensor(
                        out=zsq[:], in0=zT[dt_][:, sl].bitcast(F32),
                        in1=zT[dt_][:, sl].bitcast(F32), op=OP.mult)
                    nc.tensor.matmul(sum_ps[:], ones_col[:], zT[dt_][:, sl],
                                     start=(dt_ == 0), stop=(dt_ == DT - 1))
                    nc.tensor.matmul(sq_ps[:], ones_col[:], zsq[:],
                                     start=(dt_ == 0), stop=(dt_ == DT - 1))
                mean = ln1_pool.tile([1, 512], F32, tag="mean", name="mean")
                nc.scalar.mul(mean[:], sum_ps[:], 1.0 / D)
                msq = ln1_pool.tile([1, 512], F32, tag="msq", name="msq")
                nc.scalar.mul(msq[:], sq_ps[:], 1.0 / D)
                m2 = ln1_pool.tile([1, 512], F32, tag="m2", name="m2")
                nc.vector.tensor_mul(m2[:], mean[:], mean[:])
                var = ln1_pool.tile([1, 512], F32, tag="var", name="var")
                nc.vector.tensor_sub(var[:], msq[:], m2[:])
                std = ln1_pool.tile([1, 512], F32, tag="std", name="std")
                nc.scalar.activation(std[:], var[:], AF.Sqrt, bias=eps_sb[0:1, :])
                rstd = ln1_pool.tile([1, 512], F32, tag="rstd", name="rstd")
                nc.vector.reciprocal(rstd[:], std[:])
                mean_r = ln1_pool.tile([1, 512], F32R, tag="meanr", name="meanr")
                nc.vector.tensor_copy(mean_r[:], mean[:])
                rstd_r = ln1_pool.tile([1, 512], F32R, tag="rstdr", name="rstdr")
                nc.vector.tensor_copy(rstd_r[:], rstd[:])
                bm_ps = psBc.tile([P, 512], F32, tag="bm", name="bm")
                nc.tensor.matmul(bm_ps[:], ones_row[:], mean_r[:],
                                 start=True, stop=True)
                br_ps = psBc.tile([P, 512], F32, tag="br", name="br")
                nc.tensor.matmul(br_ps[:], ones_row[:], rstd_r[:],
                                 start=True, stop=True)
                bm = ln1_pool.tile([P, 512], F32, tag="bm_sb", name="bm_sb")
                nc.scalar.copy(bm[:], bm_ps[:])
                br = ln1_pool.tile([P, 512], F32, tag="br_sb", name="br_sb")
                nc.scalar.copy(br[:], br_ps[:])
                for dt_ in range(DT):
                    tmp = ln1_pool.tile([P, 512], F32R, tag="n1", name="n1")
                    nc.vector.scalar_tensor_tensor(
                        tmp[:], zT[dt_][:, sl].bitcast(F32),
                        1.0, bm[:], op0=OP.mult, op1=OP.subtract)
                    tmp2 = ln1_pool.tile([P, 512], F32R, tag="n2", name="n2")
                    nc.vector.scalar_tensor_tensor(
                        tmp2[:], tmp[:].bitcast(F32),
                        biases["g1_p"][:, dt_:dt_ + 1], br[:],
                        op0=OP.mult, op1=OP.mult)
                    nc.vector.tensor_scalar(
                        out=hT[dt_][:, sl], in0=tmp2[:].bitcast(F32),
                        scalar1=biases["be1_p"][:, dt_:dt_ + 1], scalar2=None,
                        op0=OP.add)

        # =============== Phase 4: FFN + residual ========================
        with tc.tile_pool(name="z2T", bufs=1) as z2T_pool:
            z2T = [z2T_pool.tile([P, T], F32R, tag=f"z2T{i}", name=f"z2T{i}")
                   for i in range(DT)]
            with tc.tile_pool(name="w1b", bufs=9) as w1b_pool, \
                 tc.tile_pool(name="w2b", bufs=9) as w2b_pool, \
                 tc.tile_pool(name="t1", bufs=12) as t1_pool, \
                 tc.tile_pool(name="o2", bufs=1) as o2_pool, \
                 tc.tile_pool(name="psF1", bufs=3, space="PSUM") as psF1, \
                 tc.tile_pool(name="psF2", bufs=3, space="PSUM") as psF2:
                out2 = [o2_pool.tile([P, T], F32, tag=f"o2{i}", name=f"o2{i}")
                        for i in range(DT)]
                for hb in range(4):              # hidden blocks of 1024
                    w1b = [w1b_pool.tile([P, D], F32R, tag="w1b", name="w1b")
                           for _ in range(DT)]
                    for i in range(DT):
                        nc.sync.dma_start(
                            out=w1b[i],
                            in_=w1[i * P:(i + 1) * P,
                                   hb * 1024:(hb + 1) * 1024])
                    w2b = [w2b_pool.tile([P, D], F32R, tag="w2b", name="w2b")
                           for _ in range(DT)]
                    for i in range(DT):
                        nc.sync.dma_start(
                            out=w2b[i],
                            in_=w2[(hb * 8 + i) * P:(hb * 8 + i + 1) * P, :])
                    for tc4 in range(T // 256):  # 4 token chunks of 256
                        tsl = slice(tc4 * 256, (tc4 + 1) * 256)
                        t1s = []
                        for i in range(DT):      # 8 hidden tiles in block
                            t1ps = psF1.tile([P, 256], F32, tag="t1ps", name="t1ps")
                            for din in range(DT):
                                nc.tensor.matmul(
                                    t1ps[:], w1b[din][:, i * P:(i + 1) * P],
                                    hT[din][:, tsl],
                                    start=(din == 0), stop=(din == DT - 1))
                            t1 = t1_pool.tile([P, 256], F32R, tag="t1", name="t1")
                            nc.scalar.activation(
                                t1[:], t1ps[:], AF.Relu,
                                bias=b1_sb[:, hb * 8 + i:hb * 8 + i + 1])
                            t1s.append(t1)
                        for dout in range(DT):
                            o2ps = psF2.tile([P, 256], F32, tag="o2ps", name="o2ps")
                            for i in range(DT):
                                nc.tensor.matmul(
                                    o2ps[:], w2b[i][:, dout * P:(dout + 1) * P],
                                    t1s[i][:],
                                    start=(i == 0), stop=(i == DT - 1))
                            if hb == 0:
                                nc.vector.tensor_copy(out2[dout][:, tsl],
                                                      o2ps[:])
                            else:
                                nc.vector.tensor_tensor(
                                    out=out2[dout][:, tsl], in0=o2ps[:],
                                    in1=out2[dout][:, tsl], op=OP.add)
                # z2 = ffn_out + b2 + h   (residual)
                for dt_ in range(DT):
                    nc.vector.scalar_tensor_tensor(
                        z2T[dt_][:], out2[dt_][:],
                        biases["b2_p"][:, dt_:dt_ + 1],
                        hT[dt_][:].bitcast(F32), op0=OP.add, op1=OP.add)

            # =============== Phase 5: transpose + LN2 + out =============
            with tc.tile_pool(name="tm", bufs=2) as tm_pool, \
                 tc.tile_pool(name="ln2", bufs=2) as ln2_pool, \
                 tc.tile_pool(name="psT5", bufs=4, space="PSUM") as psT5:
                g2_bc = ln2_pool.tile([P, D], F32, tag="g2bc", name="g2bc")
                nc.sync.dma_start(out=g2_bc, in_=bass.AP(
                    tensor=t["g2_d"], offset=0, ap=[[0, P], [1, D]]))
                be2_bc = ln2_pool.tile([P, D], F32, tag="be2bc", name="be2bc")
                nc.sync.dma_start(out=be2_bc, in_=bass.AP(
                    tensor=t["be2_d"], offset=0, ap=[[0, P], [1, D]]))
                for nt in range(DT):             # 8 token tiles of 128
                    z2 = tm_pool.tile([P, D], F32, tag="z2tm", name="z2tm")
                    for dt_ in range(DT):
                        pt = psT5.tile([P, P], F32R, tag="psT5", name="psT5")
                        nc.tensor.transpose(
                            pt[:], z2T[dt_][:, nt * P:(nt + 1) * P], ident[:])
                        nc.scalar.copy(z2[:, dt_ * P:(dt_ + 1) * P],
                                       pt[:].bitcast(F32))
                    stats = ln2_pool.tile([P, 2, 6], F32, tag="st", name="st")
                    for g in range(2):
                        nc.vector.bn_stats(out=stats[:, g, :],
                                           in_=z2[:, g * 512:(g + 1) * 512])
                    mv = ln2_pool.tile([P, 2], F32, tag="mv", name="mv")
                    nc.vector.bn_aggr(out=mv[:], in_=stats[:])
                    std = ln2_pool.tile([P, 1], F32, tag="std2", name="std2")
                    nc.scalar.activation(std[:], mv[:, 1:2], AF.Sqrt, bias=eps_sb[:])
                    rstd = ln2_pool.tile([P, 1], F32, tag="rstd2", name="rstd2")
                    nc.vector.reciprocal(rstd[:], std[:])
                    xn = ln2_pool.tile([P, D], F32, tag="xn", name="xn")
                    nc.vector.tensor_scalar(
                        out=xn[:], in0=z2[:], scalar1=mv[:, 0:1],
                        scalar2=rstd[:], op0=OP.subtract, op1=OP.mult)
                    xg = ln2_pool.tile([P, D], F32, tag="xg", name="xg")
                    nc.vector.tensor_mul(xg[:], xn[:], g2_bc[:])
                    fin = ln2_pool.tile([P, D], F32, tag="fin", name="fin")
                    nc.vector.tensor_add(fin[:], xg[:], be2_bc[:])
                    nc.sync.dma_start(out=out[nt * P:(nt + 1) * P, :],
                                      in_=fin[:])


def _pack(v, nt):
    return np.ascontiguousarray(v.reshape(nt, P).T)


def kernel(x, Wq, bq, Wk, bk, Wv, bv, Wo, bo, W1, b1, W2, b2, g1, beta1,
           g2, beta2):
    x = np.asarray(x, dtype=np.float32)
    if "nc" not in _CACHED:
        _CACHED["nc"] = _build_program()
    nc = _CACHED["nc"]

    f32 = lambda a: np.ascontiguousarray(np.asarray(a, dtype=np.float32))
    scale = 1.0 / np.sqrt(HD)
    common = {
        "wq": f32(Wq), "wk": f32(np.asarray(Wk) * scale), "wv": f32(Wv),
        "wo": f32(Wo), "w1": f32(W1), "w2": f32(W2),
        "bq_p": _pack(f32(bq), DT), "bk_p": _pack(f32(bk) * scale, DT),
        "bv_p": _pack(f32(bv), DT), "bo_p": _pack(f32(bo), DT),
        "b1_p": _pack(f32(b1), FT), "b2_p": _pack(f32(b2), DT),
        "g1_p": _pack(f32(g1), DT), "be1_p": _pack(f32(beta1), DT),
        "g2_d": f32(g2), "be2_d": f32(beta2),
        "ident_d": np.eye(P, dtype=np.float32),
        "ones_row_d": np.ones((1, P), dtype=np.float32),
        "ones_col_d": np.ones((P, 1), dtype=np.float32),
        "ones16_d": np.ones((P, H, 1), dtype=np.float32),
    }
    in_maps = []
    for c in range(N_CORES):
        b, half = c // 2, c % 2
        own = x[b, half * T:(half + 1) * T]           # [1024, 1024]
        other = x[b, (1 - half) * T:(2 - half) * T]
        xT_c = np.ascontiguousarray(
            np.concatenate([own, other], axis=0).T)   # [1024, 2048]
        in_maps.append({**common, "xT": xT_c})

    trace = bool(os.environ.get("KERNEL_TRACE"))
    res = bass_utils.run_bass_kernel_spmd(
        nc, in_maps, core_ids=list(range(N_CORES)), trace=trace)
    _CACHED["last_result"] = res

    y = np.empty((4, S, D), dtype=np.float32)
    for c in range(N_CORES):
        b, half = c // 2, c % 2
        y[b, half * T:(half + 1) * T] = res.results[c]["out"]
    return y


# revision 8
# speedup vs baseline: 1.2578x; 1.2578x over previous
"""Trainium2 Bass kernel for a dense transformer layer (attention + FFN + 2 LayerNorms).

Problem shapes: x [4, 2048, 1024], d_model=1024, heads=16 (hd=64), d_ff=4096.

Sharding: 8 cores; core c handles batch b = c//2, sequence half = c%2
(1024 query tokens).  Each core computes K/V for its batch's full 2048
tokens (duplicated across the pair — avoids any cross-core communication).
The host permutes each core's x so its own token half comes first; since
attention sums over key tokens, K/V token order is irrelevant as long as
K and V agree.

Layout strategy: activations are feature-major ("transposed": d_model on
partitions, tokens on free dim) so weight matrices serve directly as
matmul stationary operands (out = lhsT.T @ rhs).  Attention computes
S^T = K @ Q^T per head (key tokens on partitions), exp on the scalar
engine, then ctx^T = V_aug^T @ P^T where V carries a ones-column that
yields the softmax denominator for free.  All matmuls use float32r
(full-rate fp32 PE streaming, ~1e-4 relative rounding).

K^T, V (token-major) and ctx^T bounce through HBM to bound SBUF pressure.
"""

import os
import numpy as np

import concourse.bass as bass
import concourse.tile as tile
from concourse import bacc, mybir
from concourse import bass_utils

BF16 = mybir.dt.bfloat16
F32 = mybir.dt.float32
AF = mybir.ActivationFunctionType
OP = mybir.AluOpType

D = 1024          # d_model
S = 2048          # full sequence per batch
T = 1024          # query tokens per core
H = 16            # heads
HD = 64           # head dim
F = 4096          # ffn hidden
P = 128
DT = D // P       # 8 feature tiles
KT = S // P       # 16 key-token tiles
FT = F // P       # 32 hidden tiles
N_CORES = 8
EPS = 1e-5

_CACHED = {}


def _build_program():
    nc = bacc.Bacc("TRN2", target_bir_lowering=False, debug=False,
                   num_devices=N_CORES)

    tens = {}

    def di(name, shape, dtype=BF16):
        tens[name] = nc.dram_tensor(name, shape, dtype, kind="ExternalInput")

    di("xT", [D, S])
    di("wq", [D, D]); di("wk", [D, D]); di("wv", [D, D]); di("wo", [D, D])
    di("w1", [D, F]); di("w2", [F, D])
    for nm in ["bq_p", "bk_p", "bv_p", "bo_p", "b2_p", "g1_p", "be1_p"]:
        di(nm, [P, DT], F32)
    di("b1_p", [P, FT], F32)
    di("g2_d", [D], F32); di("be2_d", [D], F32)
    di("ident_d", [P, P]); di("ones_row_d", [1, P]); di("ones_col_d", [P, 1])
    di("ones16_d", [P, H, 1])
    tens["out"] = nc.dram_tensor("out", [T, D], F32, kind="ExternalOutput")

    with tile.TileContext(nc) as tc:
        _trace_kernel(nc, tc, tens)
    nc.compile()
    return nc


def _trace_kernel(nc, tc, t):
    xT, wq, wk, wv, wo, w1, w2 = (t["xT"], t["wq"], t["wk"], t["wv"], t["wo"],
                                  t["w1"], t["w2"])
    out = t["out"]

    from contextlib import ExitStack
    es = ExitStack()
    with es:
        dram = es.enter_context(tc.tile_pool(name="dram", bufs=1, space="DRAM"))
        kT_hbm = dram.tile([D, S], BF16, tag="kh", name="kh")
        v_hbm = dram.tile([S, H, HD + 1], BF16, tag="vh", name="vh")
        ctx_hbm = dram.tile([D, T], BF16, tag="ch", name="ch")

        const = es.enter_context(tc.tile_pool(name="const", bufs=1))
        ident = const.tile([P, P], BF16, tag="ident", name="ident")
        nc.sync.dma_start(out=ident, in_=t["ident_d"][:, :])
        ones_row = const.tile([1, P], BF16, tag="onesr", name="onesr")
        nc.sync.dma_start(out=ones_row, in_=t["ones_row_d"][:, :])
        ones_col = const.tile([P, 1], BF16, tag="onesc", name="onesc")
        nc.sync.dma_start(out=ones_col, in_=t["ones_col_d"][:, :])
        ones16 = const.tile([P, H, 1], BF16, tag="ones16", name="ones16")
        nc.sync.dma_start(out=ones16, in_=t["ones16_d"][:, :, :])
        biases = {}
        for name in ["bq_p", "bk_p", "bv_p", "bo_p", "b2_p", "g1_p", "be1_p"]:
            bt = const.tile([P, DT], F32, tag=name)
            nc.sync.dma_start(out=bt, in_=t[name][:, :])
            biases[name] = bt
        eps_sb = const.tile([P, 1], F32, tag="eps", name="eps")
        nc.vector.memset(eps_sb[:], EPS)
        b1_sb = const.tile([P, FT], F32, tag="b1", name="b1")
        nc.sync.dma_start(out=b1_sb, in_=t["b1_p"][:, :])
        hT_pool = es.enter_context(tc.tile_pool(name="hT", bufs=1))
        hT = [hT_pool.tile([P, T], BF16, tag=f"hT{i}", name=f"hT{i}") for i in range(DT)]

        # =============== Phase 1: QKV projections =======================
        qt_cm = tc.tile_pool(name="qt", bufs=1)
        qt_pool = qt_cm.__enter__()
        QT = [qt_pool.tile([P, T], BF16, tag=f"qt{i}", name=f"qt{i}") for i in range(DT)]

        with tc.tile_pool(name="xsb", bufs=1) as xsb_pool, \
             tc.tile_pool(name="wsb", bufs=DT) as wsb_pool, \
             tc.tile_pool(name="p1ev", bufs=2) as ev_pool, \
             tc.tile_pool(name="vsb", bufs=4) as vsb_pool, \
             tc.tile_pool(name="psA", bufs=2, space="PSUM") as psA, \
             tc.tile_pool(name="psT", bufs=4, space="PSUM") as psT:
            xsb = []
            for dt_ in range(DT):
                xt_ = xsb_pool.tile([P, S], BF16, tag=f"x{dt_}", name=f"x{dt_}")
                nc.sync.dma_start(out=xt_, in_=xT[dt_ * P:(dt_ + 1) * P, :])
                xsb.append(xt_)

            def proj_psum(ps, w_sb, dout, cols):
                for din in range(DT):
                    nc.tensor.matmul(
                        ps[:], w_sb[din][:, dout * P:(dout + 1) * P],
                        xsb[din][:, cols], start=(din == 0),
                        stop=(din == DT - 1))

            def load_w(wd):
                w_sb = [wsb_pool.tile([P, D], BF16, tag="w", name="w") for _ in range(DT)]
                for dt_ in range(DT):
                    nc.sync.dma_start(out=w_sb[dt_],
                                      in_=wd[dt_ * P:(dt_ + 1) * P, :])
                return w_sb

            # --- V projection -> transpose -> v_hbm ---------------------
            wv_sb = load_w(wv)
            for ch in range(S // 512):           # 4 chunks of 512 tokens
                vtiles = [vsb_pool.tile([P, H, HD + 1], BF16, tag="vsb", name="vsb")
                          for _ in range(4)]
                for dout in range(DT):
                    ps = psA.tile([P, 512], F32, tag="psA", name="psA")
                    proj_psum(ps, wv_sb, dout, slice(ch * 512, (ch + 1) * 512))
                    vt = ev_pool.tile([P, 512], BF16, tag="vt", name="vt")
                    nc.scalar.activation(vt[:], ps[:], AF.Identity,
                                         bias=biases["bv_p"][:, dout:dout + 1])
                    for hh in range(2):
                        h = 2 * dout + hh
                        idsl = ident[hh * HD:(hh + 1) * HD,
                                     hh * HD:(hh + 1) * HD]
                        for st in range(4):
                            pt = psT.tile([P, HD], BF16, tag="psT", name="psT")
                            nc.tensor.transpose(
                                pt[:], vt[hh * HD:(hh + 1) * HD,
                                          st * P:(st + 1) * P],
                                idsl)
                            nc.vector.tensor_copy(vtiles[st][:, h, 0:HD],
                                                  pt[:])
                for st in range(4):
                    nc.vector.tensor_copy(vtiles[st][:, :, HD:HD + 1],
                                          ones16[:, :, :])
                    kt0 = ch * 4 + st
                    nc.sync.dma_start(
                        out=v_hbm[kt0 * P:(kt0 + 1) * P, :, :],
                        in_=vtiles[st][:, :, :])

            # --- K projection -> kT_hbm ---------------------------------
            wk_sb = load_w(wk)
            for ch in range(S // 512):
                for dout in range(DT):
                    ps = psA.tile([P, 512], F32, tag="psA", name="psA")
                    proj_psum(ps, wk_sb, dout, slice(ch * 512, (ch + 1) * 512))
                    kt_ = ev_pool.tile([P, 512], BF16, tag="kt", name="kt")
                    nc.scalar.activation(kt_[:], ps[:], AF.Identity,
                                         bias=biases["bk_p"][:, dout:dout + 1])
                    nc.sync.dma_start(
                        out=kT_hbm[dout * P:(dout + 1) * P,
                                   ch * 512:(ch + 1) * 512],
                        in_=kt_[:])

            # --- Q projection (own tokens = xT cols 0:1024; resident) ---
            wq_sb = load_w(wq)
            for ch in range(T // 512):           # 2 chunks
                for dout in range(DT):
                    ps = psA.tile([P, 512], F32, tag="psA", name="psA")
                    proj_psum(ps, wq_sb, dout, slice(ch * 512, (ch + 1) * 512))
                    nc.scalar.activation(QT[dout][:, ch * 512:(ch + 1) * 512],
                                         ps[:], AF.Identity,
                                         bias=biases["bq_p"][:, dout:dout + 1])

        # =============== Phase 2: attention =============================
        with tc.tile_pool(name="kbuf", bufs=2) as kbuf_pool, \
             tc.tile_pool(name="vbuf", bufs=2 * KT) as vbuf_pool, \
             tc.tile_pool(name="pbuf", bufs=4) as pbuf_pool, \
             tc.tile_pool(name="cev", bufs=3) as cev_pool, \
             tc.tile_pool(name="psS", bufs=4, space="PSUM") as psS, \
             tc.tile_pool(name="psC", bufs=2, space="PSUM") as psC, \
             tc.tile_pool(name="psB", bufs=2, space="PSUM") as psB:
            for h in range(H):
                ksb = kbuf_pool.tile([HD, S], BF16, tag="kb", name="kb")
                nc.sync.dma_start(out=ksb, in_=kT_hbm[h * HD:(h + 1) * HD, :])
                vsb = [vbuf_pool.tile([P, HD + 1], BF16, tag="vb", name="vb")
                       for _ in range(KT)]
                for j in range(KT):
                    nc.sync.dma_start(out=vsb[j],
                                      in_=v_hbm[j * P:(j + 1) * P, h, :])
                dt_ = h // 2
                r0 = (h % 2) * HD
                for qc in range(T // 512):
                    # stage the head's Q rows at base partition 0 (matmul
                    # requires equal base partitions for lhsT and rhs)
                    qstg = cev_pool.tile([HD, 512], BF16, tag="qstg",
                                         name="qstg")
                    nc.vector.tensor_copy(
                        qstg[:], QT[dt_][r0:r0 + HD, qc * 512:(qc + 1) * 512])
                    cps = psC.tile([HD + 1, 512], F32, tag="cps", name="cps")
                    for j in range(KT):
                        sps = psS.tile([P, 512], F32, tag="sps", name="sps")
                        nc.tensor.matmul(sps[:], ksb[:, j * P:(j + 1) * P],
                                         qstg[:], start=True, stop=True)
                        pT = pbuf_pool.tile([P, 512], BF16, tag="pT", name="pT")
                        nc.scalar.activation(pT[:], sps[:], AF.Exp)
                        nc.tensor.matmul(cps[:], vsb[j][:], pT[:],
                                         start=(j == 0), stop=(j == KT - 1))
                    # normalize: reciprocal of denom row, broadcast via PE
                    recip = cev_pool.tile([1, 512], F32, tag="recip", name="recip")
                    nc.vector.reciprocal(recip[:], cps[HD:HD + 1, :])
                    recip_r = cev_pool.tile([1, 512], BF16, tag="recipr", name="recipr")
                    nc.vector.tensor_copy(recip_r[:], recip[:])
                    bps = psB.tile([HD, 512], F32, tag="bps", name="bps")
                    nc.tensor.matmul(bps[:], ones_row[:, 0:HD], recip_r[:],
                                     start=True, stop=True)
                    bcs = cev_pool.tile([HD, 512], F32, tag="bcs", name="bcs")
                    nc.scalar.copy(bcs[:], bps[:])
                    ctx_sb = cev_pool.tile([HD, 512], BF16, tag="ctxe", name="ctxe")
                    nc.vector.scalar_tensor_tensor(
                        ctx_sb[:], cps[0:HD, :], 1.0, bcs[:],
                        op0=OP.mult, op1=OP.mult)
                    nc.sync.dma_start(
                        out=ctx_hbm[h * HD:(h + 1) * HD,
                                    qc * 512:(qc + 1) * 512],
                        in_=ctx_sb[:])

        qt_cm.__exit__(None, None, None)

        # =============== Phase 3: Wo + residual + LN1 ===================
        with tc.tile_pool(name="ctxs", bufs=1) as ctxs_pool, \
             tc.tile_pool(name="xq", bufs=1) as xq_pool, \
             tc.tile_pool(name="wos", bufs=DT) as wos_pool, \
             tc.tile_pool(name="zT", bufs=1) as zT_pool, \
             tc.tile_pool(name="ln1", bufs=1) as ln1_pool, \
             tc.tile_pool(name="psW", bufs=3, space="PSUM") as psW, \
             tc.tile_pool(name="psStat", bufs=1, space="PSUM") as psStat, \
             tc.tile_pool(name="psBc", bufs=1, space="PSUM") as psBc:
            ctxs = [ctxs_pool.tile([P, T], BF16, tag=f"ctx{i}", name=f"ctx{i}")
                    for i in range(DT)]
            for dt_ in range(DT):
                nc.sync.dma_start(out=ctxs[dt_],
                                  in_=ctx_hbm[dt_ * P:(dt_ + 1) * P, :])
            xq = [xq_pool.tile([P, T], BF16, tag=f"xq{i}", name=f"xq{i}") for i in range(DT)]
            for dt_ in range(DT):
                nc.sync.dma_start(
                    out=xq[dt_],
                    in_=xT[dt_ * P:(dt_ + 1) * P, 0:T])
            wo_sb = [wos_pool.tile([P, D], BF16, tag="wo", name="wo") for _ in range(DT)]
            for dt_ in range(DT):
                nc.sync.dma_start(out=wo_sb[dt_],
                                  in_=wo[dt_ * P:(dt_ + 1) * P, :])
            zT = [zT_pool.tile([P, T], BF16, tag=f"zT{i}", name=f"zT{i}") for i in range(DT)]
            for ch in range(T // 512):
                for dout in range(DT):
                    ps = psW.tile([P, 512], F32, tag="psW", name="psW")
                    for din in range(DT):
                        nc.tensor.matmul(
                            ps[:], wo_sb[din][:, dout * P:(dout + 1) * P],
                            ctxs[din][:, ch * 512:(ch + 1) * 512],
                            start=(din == 0), stop=(din == DT - 1))
                    # z = attn_out + bo + x_resid
                    nc.vector.scalar_tensor_tensor(
                        zT[dout][:, ch * 512:(ch + 1) * 512], ps[:],
                        biases["bo_p"][:, dout:dout + 1],
                        xq[dout][:, ch * 512:(ch + 1) * 512],
                        op0=OP.add, op1=OP.add)

            # ---- LN1 (feature-major; stats over partitions via PE) -----
            for ch in range(T // 512):
                sl = slice(ch * 512, (ch + 1) * 512)
                sum_ps = psStat.tile([1, 512], F32, tag="s", name="s")
                sq_ps = psStat.tile([1, 512], F32, tag="q", name="q")
                for dt_ in range(DT):
                    zsq = ln1_pool.tile([P, 512], BF16, tag="zsq", name="zsq")
                    nc.vector.tensor_tensor(
                        out=zsq[:], in0=zT[dt_][:, sl],
                        in1=zT[dt_][:, sl], op=OP.mult)
                    nc.tensor.matmul(sum_ps[:], ones_col[:], zT[dt_][:, sl],
                                     start=(dt_ == 0), stop=(dt_ == DT - 1))
                    nc.tensor.matmul(sq_ps[:], ones_col[:], zsq[:],
                                     start=(dt_ == 0), stop=(dt_ == DT - 1))
                mean = ln1_pool.tile([1, 512], F32, tag="mean", name="mean")
                nc.scalar.mul(mean[:], sum_ps[:], 1.0 / D)
                msq = ln1_pool.tile([1, 512], F32, tag="msq", name="msq")
                nc.scalar.mul(msq[:], sq_ps[:], 1.0 / D)
                m2 = ln1_pool.tile([1, 512], F32, tag="m2", name="m2")
                nc.vector.tensor_mul(m2[:], mean[:], mean[:])
                var = ln1_pool.tile([1, 512], F32, tag="var", name="var")
                nc.vector.tensor_sub(var[:], msq[:], m2[:])
                std = ln1_pool.tile([1, 512], F32, tag="std", name="std")
                nc.scalar.activation(std[:], var[:], AF.Sqrt, bias=eps_sb[0:1, :])
                rstd = ln1_pool.tile([1, 512], F32, tag="rstd", name="rstd")
                nc.vector.reciprocal(rstd[:], std[:])
                mean_r = ln1_pool.tile([1, 512], BF16, tag="meanr", name="meanr")
                nc.vector.tensor_copy(mean_r[:], mean[:])
                rstd_r = ln1_pool.tile([1, 512], BF16, tag="rstdr", name="rstdr")
                nc.vector.tensor_copy(rstd_r[:], rstd[:])
                bm_ps = psBc.tile([P, 512], F32, tag="bm", name="bm")
                nc.tensor.matmul(bm_ps[:], ones_row[:], mean_r[:],
                                 start=True, stop=True)
                br_ps = psBc.tile([P, 512], F32, tag="br", name="br")
                nc.tensor.matmul(br_ps[:], ones_row[:], rstd_r[:],
                                 start=True, stop=True)
                bm = ln1_pool.tile([P, 512], F32, tag="bm_sb", name="bm_sb")
                nc.scalar.copy(bm[:], bm_ps[:])
                br = ln1_pool.tile([P, 512], F32, tag="br_sb", name="br_sb")
                nc.scalar.copy(br[:], br_ps[:])
                for dt_ in range(DT):
                    tmp = ln1_pool.tile([P, 512], F32, tag="n1", name="n1")
                    nc.vector.scalar_tensor_tensor(
                        tmp[:], zT[dt_][:, sl],
                        1.0, bm[:], op0=OP.mult, op1=OP.subtract)
                    tmp2 = ln1_pool.tile([P, 512], F32, tag="n2", name="n2")
                    nc.vector.scalar_tensor_tensor(
                        tmp2[:], tmp[:],
                        biases["g1_p"][:, dt_:dt_ + 1], br[:],
                        op0=OP.mult, op1=OP.mult)
                    nc.vector.tensor_scalar(
                        out=hT[dt_][:, sl], in0=tmp2[:],
                        scalar1=biases["be1_p"][:, dt_:dt_ + 1], scalar2=None,
                        op0=OP.add)

        # =============== Phase 4: FFN + residual ========================
        with tc.tile_pool(name="z2T", bufs=1) as z2T_pool:
            z2T = [z2T_pool.tile([P, T], BF16, tag=f"z2T{i}", name=f"z2T{i}")
                   for i in range(DT)]
            with tc.tile_pool(name="w1b", bufs=9) as w1b_pool, \
                 tc.tile_pool(name="w2b", bufs=9) as w2b_pool, \
                 tc.tile_pool(name="t1", bufs=12) as t1_pool, \
                 tc.tile_pool(name="o2", bufs=1) as o2_pool, \
                 tc.tile_pool(name="psF1", bufs=3, space="PSUM") as psF1, \
                 tc.tile_pool(name="psF2", bufs=3, space="PSUM") as psF2:
                out2 = [o2_pool.tile([P, T], F32, tag=f"o2{i}", name=f"o2{i}")
                        for i in range(DT)]
                for hb in range(4):              # hidden blocks of 1024
                    w1b = [w1b_pool.tile([P, D], BF16, tag="w1b", name="w1b")
                           for _ in range(DT)]
                    for i in range(DT):
                        nc.sync.dma_start(
                            out=w1b[i],
                            in_=w1[i * P:(i + 1) * P,
                                   hb * 1024:(hb + 1) * 1024])
                    w2b = [w2b_pool.tile([P, D], BF16, tag="w2b", name="w2b")
                           for _ in range(DT)]
                    for i in range(DT):
                        nc.sync.dma_start(
                            out=w2b[i],
                            in_=w2[(hb * 8 + i) * P:(hb * 8 + i + 1) * P, :])
                    for tc4 in range(T // 512):  # 2 token chunks of 512
                        tsl = slice(tc4 * 512, (tc4 + 1) * 512)
                        t1s = []
                        for i in range(DT):      # 8 hidden tiles in block
                            t1ps = psF1.tile([P, 512], F32, tag="t1ps", name="t1ps")
                            for din in range(DT):
                                nc.tensor.matmul(
                                    t1ps[:], w1b[din][:, i * P:(i + 1) * P],
                                    hT[din][:, tsl],
                                    start=(din == 0), stop=(din == DT - 1))
                            t1 = t1_pool.tile([P, 512], BF16, tag="t1", name="t1")
                            nc.scalar.activation(
                                t1[:], t1ps[:], AF.Relu,
                                bias=b1_sb[:, hb * 8 + i:hb * 8 + i + 1])
                            t1s.append(t1)
                        for dout in range(DT):
                            o2ps = psF2.tile([P, 512], F32, tag="o2ps", name="o2ps")
                            for i in range(DT):
                                nc.tensor.matmul(
                                    o2ps[:], w2b[i][:, dout * P:(dout + 1) * P],
                                    t1s[i][:],
                                    start=(i == 0), stop=(i == DT - 1))
                            if hb == 0:
                                nc.vector.tensor_copy(out2[dout][:, tsl],
                                                      o2ps[:])
                            else:
                                nc.vector.tensor_tensor(
                                    out=out2[dout][:, tsl], in0=o2ps[:],
                                    in1=out2[dout][:, tsl], op=OP.add)
                # z2 = ffn_out + b2 + h   (residual)
                for dt_ in range(DT):
                    nc.vector.scalar_tensor_tensor(
                        z2T[dt_][:], out2[dt_][:],
                        biases["b2_p"][:, dt_:dt_ + 1],
                        hT[dt_][:], op0=OP.add, op1=OP.add)

            # =============== Phase 5: transpose + LN2 + out =============
            with tc.tile_pool(name="tm", bufs=2) as tm_pool, \
                 tc.tile_pool(name="ln2", bufs=2) as ln2_pool, \
                 tc.tile_pool(name="psT5", bufs=4, space="PSUM") as psT5:
                g2_bc = ln2_pool.tile([P, D], F32, tag="g2bc", name="g2bc")
                nc.sync.dma_start(out=g2_bc, in_=bass.AP(
                    tensor=t["g2_d"], offset=0, ap=[[0, P], [1, D]]))
                be2_bc = ln2_pool.tile([P, D], F32, tag="be2bc", name="be2bc")
                nc.sync.dma_start(out=be2_bc, in_=bass.AP(
                    tensor=t["be2_d"], offset=0, ap=[[0, P], [1, D]]))
                for nt in range(DT):             # 8 token tiles of 128
                    z2 = tm_pool.tile([P, D], F32, tag="z2tm", name="z2tm")
                    for dt_ in range(DT):
                        pt = psT5.tile([P, P], BF16, tag="psT5", name="psT5")
                        nc.tensor.transpose(
                            pt[:], z2T[dt_][:, nt * P:(nt + 1) * P], ident[:])
                        nc.scalar.copy(z2[:, dt_ * P:(dt_ + 1) * P],
                                       pt[:])
                    stats = ln2_pool.tile([P, 2, 6], F32, tag="st", name="st")
                    for g in range(2):
                        nc.vector.bn_stats(out=stats[:, g, :],
                                           in_=z2[:, g * 512:(g + 1) * 512])
                    mv = ln2_pool.tile([P, 2], F32, tag="mv", name="mv")
                    nc.vector.bn_aggr(out=mv[:], in_=stats[:])
                    std = ln2_pool.tile([P, 1], F32, tag="std2", name="std2")
                    nc.scalar.activation(std[:], mv[:, 1:2], AF.Sqrt, bias=eps_sb[:])
                    rstd = ln2_pool.tile([P, 1], F32, tag="rstd2", name="rstd2")
                    nc.vector.reciprocal(rstd[:], std[:])
                    xn = ln2_pool.tile([P, D], F32, tag="xn", name="xn")
                    nc.vector.tensor_scalar(
                        out=xn[:], in0=z2[:], scalar1=mv[:, 0:1],
                        scalar2=rstd[:], op0=OP.subtract, op1=OP.mult)
                    xg = ln2_pool.tile([P, D], F32, tag="xg", name="xg")
                    nc.vector.tensor_mul(xg[:], xn[:], g2_bc[:])
                    fin = ln2_pool.tile([P, D], F32, tag="fin", name="fin")
                    nc.vector.tensor_add(fin[:], xg[:], be2_bc[:])
                    nc.sync.dma_start(out=out[nt * P:(nt + 1) * P, :],
                                      in_=fin[:])


def _pack(v, nt):
    return np.ascontiguousarray(v.reshape(nt, P).T)


def kernel(x, Wq, bq, Wk, bk, Wv, bv, Wo, bo, W1, b1, W2, b2, g1, beta1,
           g2, beta2):
    x = np.asarray(x, dtype=np.float32)
    if "nc" not in _CACHED:
        _CACHED["nc"] = _build_program()
    nc = _CACHED["nc"]

    import ml_dtypes
    bf16 = lambda a: np.ascontiguousarray(
        np.asarray(a, dtype=np.float32).astype(ml_dtypes.bfloat16))
    f32 = lambda a: np.ascontiguousarray(np.asarray(a, dtype=np.float32))
    scale = 1.0 / np.sqrt(HD)
    common = {
        "wq": bf16(Wq), "wk": bf16(np.asarray(Wk, np.float64) * scale), "wv": bf16(Wv),
        "wo": bf16(Wo), "w1": bf16(W1), "w2": bf16(W2),
        "bq_p": _pack(f32(bq), DT), "bk_p": _pack(f32(bk) * scale, DT),
        "bv_p": _pack(f32(bv), DT), "bo_p": _pack(f32(bo), DT),
        "b1_p": _pack(f32(b1), FT), "b2_p": _pack(f32(b2), DT),
        "g1_p": _pack(f32(g1), DT), "be1_p": _pack(f32(beta1), DT),
        "g2_d": f32(g2), "be2_d": f32(beta2),
        "ident_d": np.eye(P).astype(ml_dtypes.bfloat16),
        "ones_row_d": np.ones((1, P)).astype(ml_dtypes.bfloat16),
        "ones_col_d": np.ones((P, 1)).astype(ml_dtypes.bfloat16),
        "ones16_d": np.ones((P, H, 1)).astype(ml_dtypes.bfloat16),
    }
    in_maps = []
    for c in range(N_CORES):
        b, half = c // 2, c % 2
        own = x[b, half * T:(half + 1) * T]           # [1024, 1024]
        other = x[b, (1 - half) * T:(2 - half) * T]
        xT_c = np.ascontiguousarray(
            np.concatenate([own, other], axis=0).T).astype(
                ml_dtypes.bfloat16)                   # [1024, 2048]
        in_maps.append({**common, "xT": xT_c})

    trace = bool(os.environ.get("KERNEL_TRACE"))
    res = bass_utils.run_bass_kernel_spmd(
        nc, in_maps, core_ids=list(range(N_CORES)), trace=trace)
    _CACHED["last_result"] = res

    y = np.empty((4, S, D), dtype=np.float32)
    for c in range(N_CORES):
        b, half = c // 2, c % 2
        y[b, half * T:(half + 1) * T] = res.results[c]["out"]
    return y


# revision 10
# speedup vs baseline: 1.4987x; 1.1916x over previous
"""Trainium2 Bass kernel for a dense transformer layer (attention + FFN + 2 LayerNorms).

Problem shapes: x [4, 2048, 1024], d_model=1024, heads=16 (hd=64), d_ff=4096.

Sharding: 8 cores; core c handles batch b = c//2, sequence half = c%2
(1024 query tokens).  Each core computes K/V for its batch's full 2048
tokens (duplicated across the pair — avoids any cross-core communication).
The host permutes each core's x so its own token half comes first; since
attention sums over key tokens, K/V token order is irrelevant as long as
K and V agree.

Layout strategy: activations are feature-major ("transposed": d_model on
partitions, tokens on free dim) so weight matrices serve directly as
matmul stationary operands (out = lhsT.T @ rhs).  Attention computes
S^T = K @ Q^T per head (key tokens on partitions), exp on the scalar
engine, then ctx^T = V_aug^T @ P^T where V carries a ones-column that
yields the softmax denominator for free.  All matmuls use float32r
(full-rate fp32 PE streaming, ~1e-4 relative rounding).

K^T, V (token-major) and ctx^T bounce through HBM to bound SBUF pressure.
"""

import os
import numpy as np

import concourse.bass as bass
import concourse.tile as tile
from concourse import bacc, mybir
from concourse import bass_utils

BF16 = mybir.dt.bfloat16
F32 = mybir.dt.float32
AF = mybir.ActivationFunctionType
OP = mybir.AluOpType

D = 1024          # d_model
S = 2048          # full sequence per batch
T = 1024          # query tokens per core
H = 16            # heads
HD = 64           # head dim
F = 4096          # ffn hidden
P = 128
DT = D // P       # 8 feature tiles
KT = S // P       # 16 key-token tiles
FT = F // P       # 32 hidden tiles
N_CORES = 8
EPS = 1e-5

_CACHED = {}


def _build_program():
    nc = bacc.Bacc("TRN2", target_bir_lowering=False, debug=False,
                   num_devices=N_CORES)

    tens = {}

    def di(name, shape, dtype=BF16):
        tens[name] = nc.dram_tensor(name, shape, dtype, kind="ExternalInput")

    di("xT", [D, S])
    di("wq", [D, D]); di("wk", [D, D]); di("wv", [D, D]); di("wo", [D, D])
    di("w1", [D, F]); di("w2", [F, D])
    for nm in ["bq_p", "bk_p", "bv_p", "bo_p", "b2_p", "g1_p", "be1_p"]:
        di(nm, [P, DT], F32)
    di("b1_p", [P, FT], F32)
    di("g2_d", [D], F32); di("be2_d", [D], F32)
    di("ident_d", [P, P]); di("ones_row_d", [1, P]); di("ones_col_d", [P, 1])
    di("ones16_d", [P, H, 1])
    tens["out"] = nc.dram_tensor("out", [T, D], F32, kind="ExternalOutput")

    with tile.TileContext(nc) as tc:
        _trace_kernel(nc, tc, tens)
    nc.compile()
    return nc


def _trace_kernel(nc, tc, t):
    xT, wq, wk, wv, wo, w1, w2 = (t["xT"], t["wq"], t["wk"], t["wv"], t["wo"],
                                  t["w1"], t["w2"])
    out = t["out"]

    from contextlib import ExitStack
    es = ExitStack()
    with es:
        dram = es.enter_context(tc.tile_pool(name="dram", bufs=1, space="DRAM"))
        kT_hbm = dram.tile([D, S], BF16, tag="kh", name="kh")
        v_hbm = dram.tile([S, H, HD + 1], BF16, tag="vh", name="vh")
        ctx_hbm = dram.tile([D, T], BF16, tag="ch", name="ch")

        const = es.enter_context(tc.tile_pool(name="const", bufs=1))
        ident = const.tile([P, P], BF16, tag="ident", name="ident")
        nc.sync.dma_start(out=ident, in_=t["ident_d"][:, :])
        ones_row = const.tile([1, P], BF16, tag="onesr", name="onesr")
        nc.sync.dma_start(out=ones_row, in_=t["ones_row_d"][:, :])
        ones_col = const.tile([P, 1], BF16, tag="onesc", name="onesc")
        nc.sync.dma_start(out=ones_col, in_=t["ones_col_d"][:, :])
        ones16 = const.tile([P, H, 1], BF16, tag="ones16", name="ones16")
        nc.sync.dma_start(out=ones16, in_=t["ones16_d"][:, :, :])
        biases = {}
        for name in ["bq_p", "bk_p", "bv_p", "bo_p", "b2_p", "g1_p", "be1_p"]:
            bt = const.tile([P, DT], F32, tag=name)
            nc.sync.dma_start(out=bt, in_=t[name][:, :])
            biases[name] = bt
        eps_sb = const.tile([P, 1], F32, tag="eps", name="eps")
        nc.vector.memset(eps_sb[:], EPS)
        b1_sb = const.tile([P, FT], F32, tag="b1", name="b1")
        nc.sync.dma_start(out=b1_sb, in_=t["b1_p"][:, :])
        hT_pool = es.enter_context(tc.tile_pool(name="hT", bufs=1))
        hT = [hT_pool.tile([P, T], BF16, tag=f"hT{i}", name=f"hT{i}") for i in range(DT)]

        # =============== Phase 1: QKV projections =======================
        qt_cm = tc.tile_pool(name="qt", bufs=1)
        qt_pool = qt_cm.__enter__()
        QT = [qt_pool.tile([P, T], BF16, tag=f"qt{i}", name=f"qt{i}") for i in range(DT)]

        with tc.tile_pool(name="xsb", bufs=1) as xsb_pool, \
             tc.tile_pool(name="wsb", bufs=DT) as wsb_pool, \
             tc.tile_pool(name="p1ev", bufs=2) as ev_pool, \
             tc.tile_pool(name="vsb", bufs=4) as vsb_pool, \
             tc.tile_pool(name="psA", bufs=2, space="PSUM") as psA, \
             tc.tile_pool(name="psT", bufs=4, space="PSUM") as psT:
            xsb = []
            for dt_ in range(DT):
                xt_ = xsb_pool.tile([P, S], BF16, tag=f"x{dt_}", name=f"x{dt_}")
                nc.sync.dma_start(out=xt_, in_=xT[dt_ * P:(dt_ + 1) * P, :])
                xsb.append(xt_)

            def proj_psum(ps, w_sb, dout, cols):
                for din in range(DT):
                    nc.tensor.matmul(
                        ps[:], w_sb[din][:, dout * P:(dout + 1) * P],
                        xsb[din][:, cols], start=(din == 0),
                        stop=(din == DT - 1))

            def load_w(wd):
                w_sb = [wsb_pool.tile([P, D], BF16, tag="w", name="w") for _ in range(DT)]
                for dt_ in range(DT):
                    nc.sync.dma_start(out=w_sb[dt_],
                                      in_=wd[dt_ * P:(dt_ + 1) * P, :])
                return w_sb

            # --- V projection -> transpose -> v_hbm ---------------------
            wv_sb = load_w(wv)
            for ch in range(S // 512):           # 4 chunks of 512 tokens
                vtiles = [vsb_pool.tile([P, H, HD + 1], BF16, tag="vsb", name="vsb")
                          for _ in range(4)]
                for dout in range(DT):
                    ps = psA.tile([P, 512], F32, tag="psA", name="psA")
                    proj_psum(ps, wv_sb, dout, slice(ch * 512, (ch + 1) * 512))
                    vt = ev_pool.tile([P, 512], BF16, tag="vt", name="vt")
                    nc.scalar.activation(vt[:], ps[:], AF.Identity,
                                         bias=biases["bv_p"][:, dout:dout + 1])
                    for hh in range(2):
                        h = 2 * dout + hh
                        idsl = ident[hh * HD:(hh + 1) * HD,
                                     hh * HD:(hh + 1) * HD]
                        for st in range(4):
                            pt = psT.tile([P, HD], BF16, tag="psT", name="psT")
                            nc.tensor.transpose(
                                pt[:], vt[hh * HD:(hh + 1) * HD,
                                          st * P:(st + 1) * P],
                                idsl)
                            nc.vector.tensor_copy(vtiles[st][:, h, 0:HD],
                                                  pt[:])
                for st in range(4):
                    nc.vector.tensor_copy(vtiles[st][:, :, HD:HD + 1],
                                          ones16[:, :, :])
                    kt0 = ch * 4 + st
                    nc.sync.dma_start(
                        out=v_hbm[kt0 * P:(kt0 + 1) * P, :, :],
                        in_=vtiles[st][:, :, :])

            # --- K projection -> kT_hbm ---------------------------------
            wk_sb = load_w(wk)
            for ch in range(S // 512):
                for dout in range(DT):
                    ps = psA.tile([P, 512], F32, tag="psA", name="psA")
                    proj_psum(ps, wk_sb, dout, slice(ch * 512, (ch + 1) * 512))
                    kt_ = ev_pool.tile([P, 512], BF16, tag="kt", name="kt")
                    nc.scalar.activation(kt_[:], ps[:], AF.Identity,
                                         bias=biases["bk_p"][:, dout:dout + 1])
                    nc.sync.dma_start(
                        out=kT_hbm[dout * P:(dout + 1) * P,
                                   ch * 512:(ch + 1) * 512],
                        in_=kt_[:])

            # --- Q projection (own tokens = xT cols 0:1024; resident) ---
            wq_sb = load_w(wq)
            for ch in range(T // 512):           # 2 chunks
                for dout in range(DT):
                    ps = psA.tile([P, 512], F32, tag="psA", name="psA")
                    proj_psum(ps, wq_sb, dout, slice(ch * 512, (ch + 1) * 512))
                    nc.scalar.activation(QT[dout][:, ch * 512:(ch + 1) * 512],
                                         ps[:], AF.Identity,
                                         bias=biases["bq_p"][:, dout:dout + 1])

        # =============== Phase 2: attention =============================
        with tc.tile_pool(name="kbuf", bufs=2) as kbuf_pool, \
             tc.tile_pool(name="vbuf", bufs=2 * KT) as vbuf_pool, \
             tc.tile_pool(name="pbuf", bufs=4) as pbuf_pool, \
             tc.tile_pool(name="cev", bufs=3) as cev_pool, \
             tc.tile_pool(name="psS", bufs=4, space="PSUM") as psS, \
             tc.tile_pool(name="psC", bufs=2, space="PSUM") as psC, \
             tc.tile_pool(name="psB", bufs=2, space="PSUM") as psB:
            for h in range(H):
                ksb = kbuf_pool.tile([P, S], BF16, tag="kb", name="kb")
                nc.sync.dma_start(out=ksb[0:HD, :],
                                  in_=kT_hbm[h * HD:(h + 1) * HD, :])
                nc.vector.memset(ksb[HD:P, :], 0.0)
                vsb = [vbuf_pool.tile([P, HD + 1], BF16, tag="vb", name="vb")
                       for _ in range(KT)]
                for j in range(KT):
                    nc.sync.dma_start(out=vsb[j],
                                      in_=v_hbm[j * P:(j + 1) * P, h, :])
                dt_ = h // 2
                r0 = (h % 2) * HD
                for qc in range(T // 512):
                    # stage the head's Q rows at base partition 0 (matmul
                    # requires equal base partitions for lhsT and rhs)
                    qstg = cev_pool.tile([P, 512], BF16, tag="qstg",
                                         name="qstg")
                    nc.vector.tensor_copy(
                        qstg[0:HD, :],
                        QT[dt_][r0:r0 + HD, qc * 512:(qc + 1) * 512])
                    nc.vector.memset(qstg[HD:P, :], 0.0)
                    cps = psC.tile([HD + 1, 512], F32, tag="cps", name="cps")
                    for j in range(KT):
                        sps = psS.tile([P, 512], F32, tag="sps", name="sps")
                        nc.tensor.matmul(sps[:], ksb[:, j * P:(j + 1) * P],
                                         qstg[:], start=True, stop=True)
                        pT = pbuf_pool.tile([P, 512], BF16, tag="pT", name="pT")
                        nc.scalar.activation(pT[:], sps[:], AF.Exp)
                        nc.tensor.matmul(cps[:], vsb[j][:], pT[:],
                                         start=(j == 0), stop=(j == KT - 1))
                    # normalize: reciprocal of denom row, broadcast via PE
                    recip = cev_pool.tile([1, 512], F32, tag="recip", name="recip")
                    nc.vector.reciprocal(recip[:], cps[HD:HD + 1, :])
                    recip_r = cev_pool.tile([1, 512], BF16, tag="recipr", name="recipr")
                    nc.vector.tensor_copy(recip_r[:], recip[:])
                    bps = psB.tile([HD, 512], F32, tag="bps", name="bps")
                    nc.tensor.matmul(bps[:], ones_row[:, 0:HD], recip_r[:],
                                     start=True, stop=True)
                    bcs = cev_pool.tile([HD, 512], F32, tag="bcs", name="bcs")
                    nc.scalar.copy(bcs[:], bps[:])
                    ctx_sb = cev_pool.tile([HD, 512], BF16, tag="ctxe", name="ctxe")
                    nc.vector.scalar_tensor_tensor(
                        ctx_sb[:], cps[0:HD, :], 1.0, bcs[:],
                        op0=OP.mult, op1=OP.mult)
                    nc.sync.dma_start(
                        out=ctx_hbm[h * HD:(h + 1) * HD,
                                    qc * 512:(qc + 1) * 512],
                        in_=ctx_sb[:])

        qt_cm.__exit__(None, None, None)

        # =============== Phase 3: Wo + residual + LN1 ===================
        with tc.tile_pool(name="ctxs", bufs=1) as ctxs_pool, \
             tc.tile_pool(name="xq", bufs=1) as xq_pool, \
             tc.tile_pool(name="wos", bufs=DT) as wos_pool, \
             tc.tile_pool(name="zT", bufs=1) as zT_pool, \
             tc.tile_pool(name="ln1", bufs=1) as ln1_pool, \
             tc.tile_pool(name="psW", bufs=3, space="PSUM") as psW, \
             tc.tile_pool(name="psStat", bufs=1, space="PSUM") as psStat, \
             tc.tile_pool(name="psBc", bufs=1, space="PSUM") as psBc:
            ctxs = [ctxs_pool.tile([P, T], BF16, tag=f"ctx{i}", name=f"ctx{i}")
                    for i in range(DT)]
            for dt_ in range(DT):
                nc.sync.dma_start(out=ctxs[dt_],
                                  in_=ctx_hbm[dt_ * P:(dt_ + 1) * P, :])
            xq = [xq_pool.tile([P, T], BF16, tag=f"xq{i}", name=f"xq{i}") for i in range(DT)]
            for dt_ in range(DT):
                nc.sync.dma_start(
                    out=xq[dt_],
                    in_=xT[dt_ * P:(dt_ + 1) * P, 0:T])
            wo_sb = [wos_pool.tile([P, D], BF16, tag="wo", name="wo") for _ in range(DT)]
            for dt_ in range(DT):
                nc.sync.dma_start(out=wo_sb[dt_],
                                  in_=wo[dt_ * P:(dt_ + 1) * P, :])
            zT = [zT_pool.tile([P, T], BF16, tag=f"zT{i}", name=f"zT{i}") for i in range(DT)]
            for ch in range(T // 512):
                for dout in range(DT):
                    ps = psW.tile([P, 512], F32, tag="psW", name="psW")
                    for din in range(DT):
                        nc.tensor.matmul(
                            ps[:], wo_sb[din][:, dout * P:(dout + 1) * P],
                            ctxs[din][:, ch * 512:(ch + 1) * 512],
                            start=(din == 0), stop=(din == DT - 1))
                    # z = attn_out + bo + x_resid
                    nc.vector.scalar_tensor_tensor(
                        zT[dout][:, ch * 512:(ch + 1) * 512], ps[:],
                        biases["bo_p"][:, dout:dout + 1],
                        xq[dout][:, ch * 512:(ch + 1) * 512],
                        op0=OP.add, op1=OP.add)

            # ---- LN1 (feature-major; stats over partitions via PE) -----
            for ch in range(T // 512):
                sl = slice(ch * 512, (ch + 1) * 512)
                sum_ps = psStat.tile([1, 512], F32, tag="s", name="s")
                sq_ps = psStat.tile([1, 512], F32, tag="q", name="q")
                for dt_ in range(DT):
                    zsq = ln1_pool.tile([P, 512], BF16, tag="zsq", name="zsq")
                    nc.vector.tensor_tensor(
                        out=zsq[:], in0=zT[dt_][:, sl],
                        in1=zT[dt_][:, sl], op=OP.mult)
                    nc.tensor.matmul(sum_ps[:], ones_col[:], zT[dt_][:, sl],
                                     start=(dt_ == 0), stop=(dt_ == DT - 1))
                    nc.tensor.matmul(sq_ps[:], ones_col[:], zsq[:],
                                     start=(dt_ == 0), stop=(dt_ == DT - 1))
                mean = ln1_pool.tile([1, 512], F32, tag="mean", name="mean")
                nc.scalar.mul(mean[:], sum_ps[:], 1.0 / D)
                msq = ln1_pool.tile([1, 512], F32, tag="msq", name="msq")
                nc.scalar.mul(msq[:], sq_ps[:], 1.0 / D)
                m2 = ln1_pool.tile([1, 512], F32, tag="m2", name="m2")
                nc.vector.tensor_mul(m2[:], mean[:], mean[:])
                var = ln1_pool.tile([1, 512], F32, tag="var", name="var")
                nc.vector.tensor_sub(var[:], msq[:], m2[:])
                std = ln1_pool.tile([1, 512], F32, tag="std", name="std")
                nc.scalar.activation(std[:], var[:], AF.Sqrt, bias=eps_sb[0:1, :])
                rstd = ln1_pool.tile([1, 512], F32, tag="rstd", name="rstd")
                nc.vector.reciprocal(rstd[:], std[:])
                mean_r = ln1_pool.tile([1, 512], BF16, tag="meanr", name="meanr")
                nc.vector.tensor_copy(mean_r[:], mean[:])
                rstd_r = ln1_pool.tile([1, 512], BF16, tag="rstdr", name="rstdr")
                nc.vector.tensor_copy(rstd_r[:], rstd[:])
                bm_ps = psBc.tile([P, 512], F32, tag="bm", name="bm")
                nc.tensor.matmul(bm_ps[:], ones_row[:], mean_r[:],
                                 start=True, stop=True)
                br_ps = psBc.tile([P, 512], F32, tag="br", name="br")
                nc.tensor.matmul(br_ps[:], ones_row[:], rstd_r[:],
                                 start=True, stop=True)
                bm = ln1_pool.tile([P, 512], F32, tag="bm_sb", name="bm_sb")
                nc.scalar.copy(bm[:], bm_ps[:])
                br = ln1_pool.tile([P, 512], F32, tag="br_sb", name="br_sb")
                nc.scalar.copy(br[:], br_ps[:])
                for dt_ in range(DT):
                    tmp = ln1_pool.tile([P, 512], F32, tag="n1", name="n1")
                    nc.vector.scalar_tensor_tensor(
                        tmp[:], zT[dt_][:, sl],
                        1.0, bm[:], op0=OP.mult, op1=OP.subtract)
                    tmp2 = ln1_pool.tile([P, 512], F32, tag="n2", name="n2")
                    nc.vector.scalar_tensor_tensor(
                        tmp2[:], tmp[:],
                        biases["g1_p"][:, dt_:dt_ + 1], br[:],
                        op0=OP.mult, op1=OP.mult)
                    nc.vector.tensor_scalar(
                        out=hT[dt_][:, sl], in0=tmp2[:],
                        scalar1=biases["be1_p"][:, dt_:dt_ + 1], scalar2=None,
                        op0=OP.add)

        # =============== Phase 4: FFN + residual ========================
        with tc.tile_pool(name="z2T", bufs=1) as z2T_pool:
            z2T = [z2T_pool.tile([P, T], BF16, tag=f"z2T{i}", name=f"z2T{i}")
                   for i in range(DT)]
            with tc.tile_pool(name="w1b", bufs=9) as w1b_pool, \
                 tc.tile_pool(name="w2b", bufs=9) as w2b_pool, \
                 tc.tile_pool(name="t1", bufs=12) as t1_pool, \
                 tc.tile_pool(name="o2", bufs=1) as o2_pool, \
                 tc.tile_pool(name="psF1", bufs=3, space="PSUM") as psF1, \
                 tc.tile_pool(name="psF2", bufs=3, space="PSUM") as psF2:
                out2 = [o2_pool.tile([P, T], F32, tag=f"o2{i}", name=f"o2{i}")
                        for i in range(DT)]
                for hb in range(4):              # hidden blocks of 1024
                    w1b = [w1b_pool.tile([P, D], BF16, tag="w1b", name="w1b")
                           for _ in range(DT)]
                    for i in range(DT):
                        nc.sync.dma_start(
                            out=w1b[i],
                            in_=w1[i * P:(i + 1) * P,
                                   hb * 1024:(hb + 1) * 1024])
                    w2b = [w2b_pool.tile([P, D], BF16, tag="w2b", name="w2b")
                           for _ in range(DT)]
                    for i in range(DT):
                        nc.sync.dma_start(
                            out=w2b[i],
                            in_=w2[(hb * 8 + i) * P:(hb * 8 + i + 1) * P, :])
                    for tc4 in range(T // 512):  # 2 token chunks of 512
                        tsl = slice(tc4 * 512, (tc4 + 1) * 512)
                        t1s = []
                        for i in range(DT):      # 8 hidden tiles in block
                            t1ps = psF1.tile([P, 512], F32, tag="t1ps", name="t1ps")
                            for din in range(DT):
                                nc.tensor.matmul(
                                    t1ps[:], w1b[din][:, i * P:(i + 1) * P],
                                    hT[din][:, tsl],
                                    start=(din == 0), stop=(din == DT - 1))
                            t1 = t1_pool.tile([P, 512], BF16, tag="t1", name="t1")
                            nc.scalar.activation(
                                t1[:], t1ps[:], AF.Relu,
                                bias=b1_sb[:, hb * 8 + i:hb * 8 + i + 1])
                            t1s.append(t1)
                        for dout in range(DT):
                            o2ps = psF2.tile([P, 512], F32, tag="o2ps", name="o2ps")
                            for i in range(DT):
                                nc.tensor.matmul(
                                    o2ps[:], w2b[i][:, dout * P:(dout + 1) * P],
                                    t1s[i][:],
                                    start=(i == 0), stop=(i == DT - 1))
                            if hb == 0:
                                nc.vector.tensor_copy(out2[dout][:, tsl],
                                                      o2ps[:])
                            else:
                                nc.vector.tensor_tensor(
                                    out=out2[dout][:, tsl], in0=o2ps[:],
                                    in1=out2[dout][:, tsl], op=OP.add)
                # z2 = ffn_out + b2 + h   (residual)
                for dt_ in range(DT):
                    nc.vector.scalar_tensor_tensor(
                        z2T[dt_][:], out2[dt_][:],
                        biases["b2_p"][:, dt_:dt_ + 1],
                        hT[dt_][:], op0=OP.add, op1=OP.add)

            # =============== Phase 5: transpose + LN2 + out =============
            with tc.tile_pool(name="tm", bufs=2) as tm_pool, \
                 tc.tile_pool(name="ln2", bufs=2) as ln2_pool, \
                 tc.tile_pool(name="psT5", bufs=4, space="PSUM") as psT5:
                g2_bc = ln2_pool.tile([P, D], F32, tag="g2bc", name="g2bc")
                nc.sync.dma_start(out=g2_bc, in_=bass.AP(
                    tensor=t["g2_d"], offset=0, ap=[[0, P], [1, D]]))
                be2_bc = ln2_pool.tile([P, D], F32, tag="be2bc", name="be2bc")
                nc.sync.dma_start(out=be2_bc, in_=bass.AP(
                    tensor=t["be2_d"], offset=0, ap=[[0, P], [1, D]]))
                for nt in range(DT):             # 8 token tiles of 128
                    z2 = tm_pool.tile([P, D], F32, tag="z2tm", name="z2tm")
                    for dt_ in range(DT):
                        pt = psT5.tile([P, P], BF16, tag="psT5", name="psT5")
                        nc.tensor.transpose(
                            pt[:], z2T[dt_][:, nt * P:(nt + 1) * P], ident[:])
                        nc.scalar.copy(z2[:, dt_ * P:(dt_ + 1) * P],
                                       pt[:])
                    stats = ln2_pool.tile([P, 2, 6], F32, tag="st", name="st")
                    for g in range(2):
                        nc.vector.bn_stats(out=stats[:, g, :],
                                           in_=z2[:, g * 512:(g + 1) * 512])
                    mv = ln2_pool.tile([P, 2], F32, tag="mv", name="mv")
                    nc.vector.bn_aggr(out=mv[:], in_=stats[:])
                    std = ln2_pool.tile([P, 1], F32, tag="std2", name="std2")
                    nc.scalar.activation(std[:], mv[:, 1:2], AF.Sqrt, bias=eps_sb[:])
                    rstd = ln2_pool.tile([P, 1], F32, tag="rstd2", name="rstd2")
                    nc.vector.reciprocal(rstd[:], std[:])
                    xn = ln2_pool.tile([P, D], F32, tag="xn", name="xn")
                    nc.vector.tensor_scalar(
                        out=xn[:], in0=z2[:], scalar1=mv[:, 0:1],
                        scalar2=rstd[:], op0=OP.subtract, op1=OP.mult)
                    xg = ln2_pool.tile([P, D], F32, tag="xg", name="xg")
                    nc.vector.tensor_mul(xg[:], xn[:], g2_bc[:])
                    fin = ln2_pool.tile([P, D], F32, tag="fin", name="fin")
                    nc.vector.tensor_add(fin[:], xg[:], be2_bc[:])
                    nc.sync.dma_start(out=out[nt * P:(nt + 1) * P, :],
                                      in_=fin[:])


def _pack(v, nt):
    return np.ascontiguousarray(v.reshape(nt, P).T)


def kernel(x, Wq, bq, Wk, bk, Wv, bv, Wo, bo, W1, b1, W2, b2, g1, beta1,
           g2, beta2):
    x = np.asarray(x, dtype=np.float32)
    if "nc" not in _CACHED:
        _CACHED["nc"] = _build_program()
    nc = _CACHED["nc"]

    import ml_dtypes
    bf16 = lambda a: np.ascontiguousarray(
        np.asarray(a, dtype=np.float32).astype(ml_dtypes.bfloat16))
    f32 = lambda a: np.ascontiguousarray(np.asarray(a, dtype=np.float32))
    scale = 1.0 / np.sqrt(HD)
    common = {
        "wq": bf16(Wq), "wk": bf16(np.asarray(Wk, np.float64) * scale), "wv": bf16(Wv),
        "wo": bf16(Wo), "w1": bf16(W1), "w2": bf16(W2),
        "bq_p": _pack(f32(bq), DT), "bk_p": _pack(f32(bk) * scale, DT),
        "bv_p": _pack(f32(bv), DT), "bo_p": _pack(f32(bo), DT),
        "b1_p": _pack(f32(b1), FT), "b2_p": _pack(f32(b2), DT),
        "g1_p": _pack(f32(g1), DT), "be1_p": _pack(f32(beta1), DT),
        "g2_d": f32(g2), "be2_d": f32(beta2),
        "ident_d": np.eye(P).astype(ml_dtypes.bfloat16),
        "ones_row_d": np.ones((1, P)).astype(ml_dtypes.bfloat16),
        "ones_col_d": np.ones((P, 1)).astype(ml_dtypes.bfloat16),
        "ones16_d": np.ones((P, H, 1)).astype(ml_dtypes.bfloat16),
    }
    in_maps = []
    for c in range(N_CORES):
        b, half = c // 2, c % 2
        own = x[b, half * T:(half + 1) * T]           # [1024, 1024]
        other = x[b, (1 - half) * T:(2 - half) * T]
        xT_c = np.ascontiguousarray(
            np.concatenate([own, other], axis=0).T).astype(
                ml_dtypes.bfloat16)                   # [1024, 2048]
        in_maps.append({**common, "xT": xT_c})

    trace = bool(os.environ.get("KERNEL_TRACE"))
    res = bass_utils.run_bass_kernel_spmd(
        nc, in_maps, core_ids=list(range(N_CORES)), trace=trace)
    _CACHED["last_result"] = res

    y = np.empty((4, S, D), dtype=np.float32)
    for c in range(N_CORES):
        b, half = c // 2, c % 2
        y[b, half * T:(half + 1) * T] = res.results[c]["out"]
    return y
